# revision 15
# baseline (speedup 1.0000x reference)
"""Trainium2 Bass kernel: Conformer block (B=8, N=512, DIM=512, H=8, DH=64, FF=2048, CIN=1024, K=31).

Sharding: pure data-parallel over batch — each of the 8 NeuronCores processes one
batch item with the full weight set (no collectives).

Layout: activations are kept FEATURE-major ([feature, time] = x.T) on chip so that
chained matmuls need no transposes.  LayerNorm reductions over features become
ones-vector matmuls on the PE.

v2 changes over the 400us baseline:
  * all large weights + matmul operands in bf16 (halves HBM traffic; PE rate same)
  * qr shift-gather scratch in bf16; writes on the ACT HWDGE ring, reads on the
    GPSIMD SWDGE ring with a bf16->fp32 cast during DMA (Tile inserts the
    DRAM RAW dep automatically, verified)
  * swish via one ACT Silu op (ff + BN), GLU sigmoid via Tanh — drops ~20 DVE ops
    and keeps the ACT table-set alternation to exp/ln <-> silu/tanh only
  * depthwise-conv diagonal stationaries built on-chip from the compact [P, CT*KW]
    weights with DVE tensor_scalar (kills the 8.1 MB dwdiag DMA)
  * PE warm-up matmuls at start + keep-warm filler matmuls across the LayerNorm
    serial chains so the HAM clock gate stays at K=8/8 (baseline spent 225us at
    half clock)
  * k-proj bias dropped (softmax-invariant), v bias folded into the out-proj bias
"""

import sys

for _p in ("/opt/trn_rl_repo", "/root/.axon_site/_ro/trn_rl_repo"):
    if _p not in sys.path:
        sys.path.insert(0, _p)

import numpy as np

B, N, DIM, H, DH, MULT, EXP, KW, MAXP = 8, 512, 512, 8, 64, 4, 2, 31, 512
INNER = H * DH
FF = DIM * MULT
CIN = DIM * EXP
EPS = 1e-5
P = 128
DT = DIM // P      # 4  feature tiles of the residual stream
FT = FF // P       # 16 ff hidden tiles
CT = CIN // P      # 8  conv channel tiles
NCORES = 8
PAD = KW - 1       # 30 causal pad


def build(split_waits=True):
    """Build the single-core Bass module (SPMD: same NEFF on all 8 cores)."""
    import concourse.bass as bass
    import concourse.mybir as mybir
    import concourse.tile as tile

    F32 = mybir.dt.float32
    F32R = mybir.dt.float32r
    BF16 = mybir.dt.bfloat16
    AF = mybir.ActivationFunctionType
    AL = mybir.AluOpType

    nc = bass.Bass()

    # ---------------- I/O ----------------
    xT_d = nc.dram_tensor("xT", [DIM, N], F32R, kind="ExternalInput")
    w1_d = nc.dram_tensor("w1", [DIM, FF], BF16, kind="ExternalInput")
    b1_d = nc.dram_tensor("b1", [P, FT], F32, kind="ExternalInput")
    w2_d = nc.dram_tensor("w2", [FF, DIM], BF16, kind="ExternalInput")
    b2_d = nc.dram_tensor("b2", [P, DT], F32, kind="ExternalInput")
    wq_d = nc.dram_tensor("wq", [DIM, INNER], BF16, kind="ExternalInput")
    bq_d = nc.dram_tensor("bq", [P, DT], F32, kind="ExternalInput")
    wk_d = nc.dram_tensor("wk", [DIM, INNER], BF16, kind="ExternalInput")
    wv_d = nc.dram_tensor("wv", [DIM, INNER], BF16, kind="ExternalInput")
    wo_d = nc.dram_tensor("wo", [INNER, DIM], BF16, kind="ExternalInput")
    bo_d = nc.dram_tensor("bo", [P, DT], F32, kind="ExternalInput")
    relT_d = nc.dram_tensor("relT", [P, 2 * MAXP + 1], BF16, kind="ExternalInput")
    c1_d = nc.dram_tensor("c1", [DIM, 2 * CIN], BF16, kind="ExternalInput")
    c1a_d = nc.dram_tensor("c1a", [P, CT], F32, kind="ExternalInput")
    c1gh_d = nc.dram_tensor("c1gh", [P, CT], F32, kind="ExternalInput")
    dww_d = nc.dram_tensor("dww", [P, CT * KW], F32, kind="ExternalInput")
    bns_d = nc.dram_tensor("bns", [P, CT], F32, kind="ExternalInput")
    bnt_d = nc.dram_tensor("bnt", [P, CT], F32, kind="ExternalInput")
    c2_d = nc.dram_tensor("c2", [CIN, DIM], BF16, kind="ExternalInput")
    c2b_d = nc.dram_tensor("c2b", [P, DT], F32, kind="ExternalInput")
    w3_d = nc.dram_tensor("w3", [DIM, FF], BF16, kind="ExternalInput")
    b3_d = nc.dram_tensor("b3", [P, FT], F32, kind="ExternalInput")
    w4_d = nc.dram_tensor("w4", [FF, DIM], BF16, kind="ExternalInput")
    b4_d = nc.dram_tensor("b4", [P, DT], F32, kind="ExternalInput")
    png_d = nc.dram_tensor("png", [P, DT], F32, kind="ExternalInput")
    pnb_d = nc.dram_tensor("pnb", [P, DT], F32, kind="ExternalInput")
    antid_d = nc.dram_tensor("antid", [P, P], F32R, kind="ExternalInput")
    antidb_d = nc.dram_tensor("antidb", [P, P], BF16, kind="ExternalInput")
    onesf_d = nc.dram_tensor("onesf", [P, P], F32R, kind="ExternalInput")

    outT_d = nc.dram_tensor("outT", [DIM, N], F32, kind="ExternalOutput")

    QRW = 2 * MAXP + 1  # 1025 scratch row width
    qr_d = nc.dram_tensor("qr_scratch", [H, N, QRW], BF16, kind="Internal")

    def r32(ap):
        return ap.bitcast(F32R)

    with tile.TileContext(nc) as tc:
        with (
            nc.allow_low_precision(reason="bf16 matmul feeds"),
            tc.tile_pool(name="cst", bufs=1) as cst,
            tc.tile_pool(name="sb", bufs=2) as sb,
            tc.tile_pool(name="ps", bufs=2, space="PSUM") as psp,
        ):

            # ---------------- constants (ident/ones first: warmup needs them) ----------------
            ident = cst.tile([P, P], F32R, tag="ident")
            nc.sync.dma_start(ident[:, :], antid_d[:, :])
            ones_full = cst.tile([P, P], F32R, tag="ones_full")
            nc.sync.dma_start(ones_full[:, :], onesf_d[:, :])
            identb = cst.tile([P, P], BF16, tag="identb")
            nc.sync.dma_start(identb[:, :], antidb_d[:, :])
            # persistent 512-col filler source (copy of x tile 0) -- loaded
            # before x so the warm-up can start as early as possible
            fsrc = cst.tile([P, N], F32R, tag="fsrc")
            nc.sync.dma_start(fsrc[:, :], xT_d[0:P, :])

            # ---------------- load x (already transposed on host) ----------------
            xs = []
            for mt in range(DT):
                xt = sb.tile([P, N], F32R, tag="x", bufs=7)
                nc.sync.dma_start(xt[:, :], xT_d[mt * P:(mt + 1) * P, :])
                xs.append(xt)

            relT = cst.tile([P, QRW], BF16, tag="relT")
            nc.sync.dma_start(relT[:, :], relT_d[:, :])
            dww = cst.tile([P, CT * KW], F32, tag="dww")
            nc.sync.dma_start(dww[:, :], dww_d[:, :])
            b1t = cst.tile([P, FT], F32, tag="b1t")
            nc.sync.dma_start(b1t[:, :], b1_d[:, :])
            b2t = cst.tile([P, DT], F32, tag="b2t")
            nc.sync.dma_start(b2t[:, :], b2_d[:, :])
            bqt = cst.tile([P, DT], F32, tag="bqt")
            nc.sync.dma_start(bqt[:, :], bq_d[:, :])
            bot = cst.tile([P, DT], F32, tag="bot")
            nc.sync.dma_start(bot[:, :], bo_d[:, :])
            c1at = cst.tile([P, CT], F32, tag="c1at")
            nc.sync.dma_start(c1at[:, :], c1a_d[:, :])
            c1ght = cst.tile([P, CT], F32, tag="c1ght")
            nc.sync.dma_start(c1ght[:, :], c1gh_d[:, :])
            bnst = cst.tile([P, CT], F32, tag="bnst")
            nc.sync.dma_start(bnst[:, :], bns_d[:, :])
            bntt = cst.tile([P, CT], F32, tag="bntt")
            nc.sync.dma_start(bntt[:, :], bnt_d[:, :])
            c2bt = cst.tile([P, DT], F32, tag="c2bt")
            nc.sync.dma_start(c2bt[:, :], c2b_d[:, :])
            b3t = cst.tile([P, FT], F32, tag="b3t")
            nc.sync.dma_start(b3t[:, :], b3_d[:, :])
            b4t = cst.tile([P, DT], F32, tag="b4t")
            nc.sync.dma_start(b4t[:, :], b4_d[:, :])
            pngt = cst.tile([P, DT], F32, tag="pngt")
            nc.sync.dma_start(pngt[:, :], png_d[:, :])
            pnbt = cst.tile([P, DT], F32, tag="pnbt")
            nc.sync.dma_start(pnbt[:, :], pnb_d[:, :])

            # ---------------- PE warm-up + keep-warm filler ----------------
            def fill(n, tag="mm"):
                """Dummy 512-col matmuls on constant tiles: keeps the HAM clock
                gate at K=8/8 across serial (non-PE) dependency chains.  Each
                is ~213ns warm / ~427ns cold of PE-busy.  The PE queue is
                in-order, so callers interleave these *between* stalling
                instructions; tag picks a PSUM bank that is free in that
                phase ('mm' during LN, 's1' in the attention head loop)."""
                for _ in range(n):
                    ft = psp.tile([P, N], F32, tag=tag,
                                  bufs=(2 if tag == "mm" else 1))
                    nc.tensor.matmul(ft[:, :], ident[:, :], fsrc[:, :],
                                     start=True, stop=True)

            # ~4us of PE busy at cold clock: HAM un-throttles before ff1.
            fill(10)

            # pre-build the first two depthwise diagonal stationaries (DVE is
            # idle at kernel start; dg pool bufs=2)
            dg_tiles = {}

            def build_dg(ct):
                dg = sb.tile([P, KW * P], BF16, tag="dg", bufs=2)
                for k in range(KW):
                    nc.vector.tensor_scalar(
                        out=dg[:, k * P:(k + 1) * P], in0=ident[:, :],
                        scalar1=dww[:, ct * KW + k:ct * KW + k + 1], scalar2=None,
                        op0=AL.mult)
                dg_tiles[ct] = dg

            build_dg(0)
            build_dg(1)

            # ---------------- helpers ----------------
            def layer_norm_rc(xin):
                """LN stats over the partition (feature) axis.

                Returns m_b (mean) and r_b (rsqrt(var+eps)) [128, N] tiles.
                Fillers interleave with the stats matmuls (which trickle in as
                the residual adds land) and cover the scalar chain."""
                ps_sum = psp.tile([P, N], F32, tag="s1", bufs=1)
                for kt in range(DT):
                    fill(1)
                    nc.tensor.matmul(ps_sum[:, :], ones_full[:, :], xin[kt][:, :],
                                     start=(kt == 0), stop=(kt == DT - 1))
                m_b = sb.tile([P, N], F32, tag="mtile", bufs=1)
                nc.vector.tensor_scalar(out=m_b[:, :], in0=ps_sum[:, :],
                                        scalar1=1.0 / DIM, scalar2=None, op0=AL.mult)
                # -m^2 + eps, off the critical path (Pool rejects TensorScalarPtr)
                nm2 = sb.tile([P, N], F32, tag="tmp", bufs=3)
                nc.vector.scalar_tensor_tensor(nm2[:, :], m_b[:, :], -1.0, m_b[:, :],
                                               AL.mult, AL.mult)
                nm2e = sb.tile([P, N], F32, tag="tmp", bufs=3)
                nc.vector.tensor_scalar(out=nm2e[:, :], in0=nm2[:, :],
                                        scalar1=EPS, scalar2=None, op0=AL.add)
                ps_sq = psp.tile([P, N], F32, tag="s2", bufs=1)
                for kt in range(DT):
                    xsq = sb.tile([P, N], F32R, tag="tmp", bufs=3)
                    if kt < 2:
                        nc.scalar.square(xsq[:, :], xin[kt][:, :])
                    else:
                        nc.gpsimd.tensor_mul(xsq[:, :], xin[kt][:, :], xin[kt][:, :])
                    fill(1)
                    nc.tensor.matmul(ps_sq[:, :], ones_full[:, :], xsq[:, :],
                                     start=(kt == 0), stop=(kt == DT - 1))
                # var + eps = sumsq/DIM - m^2 + eps in one op
                veps = sb.tile([P, N], F32, tag="tmp", bufs=3)
                nc.vector.scalar_tensor_tensor(veps[:, :], ps_sq[:, :], 1.0 / DIM,
                                               nm2e[:, :], AL.mult, AL.add)
                lnv = sb.tile([P, N], F32, tag="tmp", bufs=3)
                nc.scalar.activation(lnv[:, :], veps[:, :], AF.Ln)
                r_b = sb.tile([P, N], F32, tag="r_b", bufs=2)
                nc.scalar.activation(r_b[:, :], lnv[:, :], AF.Exp, scale=-0.5)
                fill(8)
                return m_b, r_b

            def ln_apply(xin, m_b, r_b):
                """z = (x - m) * r  in bf16; split across DVE and GPSIMD."""
                zs = []
                for kt in range(DT):
                    eng = nc.vector if kt < 2 else nc.gpsimd
                    t = sb.tile([P, N], F32, tag="lnt", bufs=2)
                    eng.tensor_sub(t[:, :], xin[kt][:, :], m_b[:, :])
                    z = sb.tile([P, N], BF16, tag="z", bufs=4)
                    eng.tensor_mul(z[:, :], t[:, :], r_b[:, :])
                    zs.append(z)
                return zs

            def residual(ps_list, bias_t, xin):
                """xo[mt] = ps[mt] + bias[mt] + xin[mt]; kt 0/1 via DVE stt,
                kt 2/3 via ACT copy-bias + GPSIMD add so the four tiles land
                ~2x faster (the next LN's stats matmuls are gated on them)."""
                xo = []
                for mt in range(DT):
                    t = sb.tile([P, N], F32R, tag="x", bufs=7)
                    if mt < 2:
                        nc.vector.scalar_tensor_tensor(t[:, :], ps_list[mt][:, :],
                                                       bias_t[:, mt:mt + 1],
                                                       xin[mt][:, :],
                                                       AL.add, AL.add)
                    else:
                        u = sb.tile([P, N], F32, tag="resu", bufs=2)
                        nc.scalar.activation(u[:, :], ps_list[mt][:, :],
                                             AF.Identity,
                                             bias=bias_t[:, mt:mt + 1], scale=1.0)
                        nc.gpsimd.tensor_add(t[:, :], u[:, :], xin[mt][:, :])
                    xo.append(t)
                return xo

            def ff_block(xin, w_d, bt, w2bf_d, b2tt):
                """x + 0.5*ff(LN(x)); returns new residual tiles."""
                m_b, r_b = layer_norm_rc(xin)
                zs = ln_apply(xin, m_b, r_b)
                # h = silu(z @ w1 + b1), mt-outer with half-width weight tiles
                h1s = []
                for half in range(2):
                    wts = []
                    for kt in range(DT):
                        wt = sb.tile([P, FF // 2], BF16, tag="wbig", bufs=5)
                        nc.sync.dma_start(
                            wt[:, :], w_d[kt * P:(kt + 1) * P,
                                          half * (FF // 2):(half + 1) * (FF // 2)])
                        wts.append(wt)
                    for mh in range(FT // 2):
                        mt = half * (FT // 2) + mh
                        ph = psp.tile([P, N], F32, tag="acc", bufs=4)
                        for kt in range(DT):
                            nc.tensor.matmul(ph[:, :],
                                             wts[kt][:, mh * P:(mh + 1) * P],
                                             zs[kt][:, :],
                                             start=(kt == 0), stop=(kt == DT - 1))
                        hs = sb.tile([P, N], BF16, tag="h1s", bufs=16)
                        nc.scalar.activation(hs[:, :], ph[:, :], AF.Silu,
                                             bias=bt[:, mt:mt + 1], scale=1.0)
                        h1s.append(hs)
                # y = h @ w2 (bf16), kt-outer with 4 psum accumulators
                pys = [psp.tile([P, N], F32, tag="acc", bufs=4, name=f"pys{i}") for i in range(DT)]
                for kt in range(FT):
                    wt = sb.tile([P, DIM], BF16, tag="wsmb", bufs=6)
                    nc.sync.dma_start(wt[:, :], w2bf_d[kt * P:(kt + 1) * P, :])
                    for mt in range(DT):
                        nc.tensor.matmul(pys[mt][:, :], wt[:, mt * P:(mt + 1) * P],
                                         h1s[kt][:, :],
                                         start=(kt == 0), stop=(kt == FT - 1))
                return residual(pys, b2tt, xin)

            # ================= ff1 =================
            x1 = ff_block(xs, w1_d, b1t, w2_d, b2t)

            # ================= attention =================
            m_b, r_b = layer_norm_rc(x1)
            zs = ln_apply(x1, m_b, r_b)

            def proj(w_dram, bias_t, tag):
                wts = []
                for kt in range(DT):
                    wt = sb.tile([P, INNER], BF16, tag="wsm", bufs=4)
                    nc.sync.dma_start(wt[:, :], w_dram[kt * P:(kt + 1) * P, :])
                    wts.append(wt)
                outs = []
                for mt in range(DT):
                    pq = psp.tile([P, N], F32, tag="mm", bufs=2)
                    for kt in range(DT):
                        nc.tensor.matmul(pq[:, :], wts[kt][:, mt * P:(mt + 1) * P],
                                         zs[kt][:, :],
                                         start=(kt == 0), stop=(kt == DT - 1))
                    qt = sb.tile([P, N], BF16, tag=tag, bufs=4)
                    if bias_t is not None:
                        nc.vector.tensor_scalar(out=qt[:, :], in0=pq[:, :],
                                                scalar1=bias_t[:, mt:mt + 1],
                                                scalar2=None, op0=AL.add)
                    else:
                        nc.vector.tensor_copy(qt[:, :], pq[:, :])
                    outs.append(qt)
                return outs

            qTs = proj(wq_d, bqt, "qT")

            # preload k/v weights (wsm pool holds all 8 + wo later)
            wkts, wvts = [], []
            for kt in range(DT):
                wt = sb.tile([P, INNER], BF16, tag="wsm", bufs=9)
                nc.sync.dma_start(wt[:, :], wk_d[kt * P:(kt + 1) * P, :])
                wkts.append(wt)
            for kt in range(DT):
                wt = sb.tile([P, INNER], BF16, tag="wsm", bufs=9)
                nc.sync.dma_start(wt[:, :], wv_d[kt * P:(kt + 1) * P, :])
                wvts.append(wt)

            kTs = [None] * DT
            vext = [None] * DT

            def kv_job(idx):
                """One k- or v-projection unit, interleaved into the qr loop to
                keep the PE dense while the qr bounce copies/DMAs drain."""
                if idx < DT:
                    mt = idx
                    pq = psp.tile([P, N], F32, tag="mm", bufs=2)
                    for kt in range(DT):
                        nc.tensor.matmul(pq[:, :],
                                         wkts[kt][:, mt * P:(mt + 1) * P],
                                         zs[kt][:, :],
                                         start=(kt == 0), stop=(kt == DT - 1))
                    ktile = sb.tile([P, N], BF16, tag="kT", bufs=4)
                    nc.vector.tensor_copy(ktile[:, :], pq[:, :])
                    kTs[mt] = ktile
                else:
                    jt = idx - DT
                    pv = psp.tile([P, N], F32, tag="mm", bufs=2)
                    for kt in range(DT):
                        nc.tensor.matmul(pv[:, :], zs[kt][:, jt * P:(jt + 1) * P],
                                         wvts[kt][:, :],
                                         start=(kt == 0), stop=(kt == DT - 1))
                    vx = sb.tile([P, H * 2 * DH], BF16, tag="vext", bufs=4)
                    vw = vx[:, 0:H * 2 * DH].rearrange("p (h c) -> p h c", c=2 * DH)
                    nc.vector.tensor_copy(
                        vw[:, :, 0:DH],
                        pv[:, :].rearrange("p (h d) -> p h d", h=H))
                    nc.gpsimd.memset(vw[:, :, DH:2 * DH], 1.0)
                    vext[jt] = vx

            # qr = q @ relT -> bf16 DRAM scratch (write on ACT HWDGE ring);
            # only needs q; k/v projection matmuls interleave every 4 tiles.
            unit = 0
            for h in range(H):
                hb = (h % 2) * DH
                for it in range(DT):
                    lq = qTs[h // 2][hb:hb + DH, it * P:(it + 1) * P]
                    cr0 = 3 * P - P * it
                    pq1 = psp.tile([P, 320], F32, tag="acc", bufs=4)
                    nc.tensor.matmul(pq1[:, :], lq,
                                     relT[hb:hb + DH, cr0:cr0 + 320],
                                     start=True, stop=True)
                    pq2 = psp.tile([P, 320], F32, tag="acc", bufs=4)
                    nc.tensor.matmul(pq2[:, :], lq,
                                     relT[hb:hb + DH, cr0 + 320:cr0 + 640],
                                     start=True, stop=True)
                    qt = sb.tile([P, 640], BF16, tag="qt", bufs=3)
                    nc.scalar.copy(qt[:, 0:320], pq1[:, :])
                    nc.vector.tensor_copy(qt[:, 320:640], pq2[:, :])
                    nc.sync.dma_start(
                        qr_d[h, it * P:(it + 1) * P, cr0:cr0 + 640], qt[:, :])
                    if unit % 4 == 3:
                        kv_job(unit // 4)
                    unit += 1

            # re-warm insurance before the ACT-paced head loop
            fill(8)

            # scores (transposed), softmax over partitions, attn @ v.
            # rel[j, i] tiles come straight from the bf16 scratch via xbar
            # transpose-DMA (element (i, j) of head h lives at flat
            # H0 + 1024*i + 512 + j), then accumulate into the score PSUM
            # with one identity copy-matmul per tile — no PE transposes.
            oTs = [sb.tile([P, N], BF16, tag="oT", bufs=4, name=f"oTs{i}") for i in range(DT)]
            for h in range(H):
                hb = (h % 2) * DH
                fill(2, tag="s1")
                rTs = []
                for jt in range(DT):
                    rT = sb.tile([P, N], BF16, tag="rel", bufs=6)
                    src = bass.AP(qr_d, h * N * QRW + 4 * P + jt * P,
                                  [[QRW - 1, N], [1, P]])
                    nc.sync.dma_start(rT[:, :], src, transpose=True)
                    rTs.append(rT)
                exps = []
                for jt in range(DT):
                    pss = psp.tile([P, N], F32, tag="acc", bufs=4)
                    nc.tensor.matmul(pss[:, :],
                                     kTs[h // 2][hb:hb + DH, jt * P:(jt + 1) * P],
                                     qTs[h // 2][hb:hb + DH, :],
                                     start=True, stop=False)
                    nc.tensor.matmul(pss[:, :], identb[:, :], rTs[jt][:, :],
                                     start=False, stop=True)
                    e = sb.tile([P, N], BF16, tag="exp", bufs=5)
                    nc.scalar.activation(e[:, :], pss[:, :], AF.Exp)
                    exps.append(e)
                po = psp.tile([P, N], F32, tag="mm", bufs=2)
                for jt in range(DT):
                    nc.tensor.matmul(po[:, :],
                                     vext[jt][:, h * 2 * DH:(h + 1) * 2 * DH],
                                     exps[jt][:, :],
                                     start=(jt == 0), stop=(jt == DT - 1))
                lnd = sb.tile([DH, N], F32, tag="dwt", bufs=3)
                nc.scalar.activation(lnd[:, :], po[DH:2 * DH, :], AF.Ln)
                rb = sb.tile([DH, N], F32, tag="dwt", bufs=3)
                nc.scalar.activation(rb[:, :], lnd[:, :], AF.Exp, scale=-1.0)
                nc.vector.tensor_mul(oTs[h // 2][hb:hb + DH, :], po[0:DH, :],
                                     rb[:, :])

            # out-projection + residual
            wots = []
            for kt in range(DT):
                wt = sb.tile([P, DIM], BF16, tag="wsm", bufs=4)
                nc.sync.dma_start(wt[:, :], wo_d[kt * P:(kt + 1) * P, :])
                wots.append(wt)
            pas = [psp.tile([P, N], F32, tag="acc", bufs=4, name=f"pas{i}") for i in range(DT)]
            for kt in range(DT):
                for mt in range(DT):
                    nc.tensor.matmul(pas[mt][:, :], wots[kt][:, mt * P:(mt + 1) * P],
                                     oTs[kt][:, :],
                                     start=(kt == 0), stop=(kt == DT - 1))
            x2 = residual(pas, bot, x1)
            fill(4, tag="s1")

            # bf16 copies of x2 for the conv matmuls (GPSIMD; keeps residual fp32)
            x2b = []
            for mt in range(DT):
                t = sb.tile([P, N], BF16, tag="x2b", bufs=4)
                nc.gpsimd.tensor_copy(t[:, :], x2[mt][:, :])
                x2b.append(t)

            # ================= conv module =================
            glus = []
            for half in range(2):
                c1ts = []
                for kt in range(DT):
                    wt = sb.tile([P, CIN], BF16, tag="wbig", bufs=5)
                    nc.sync.dma_start(
                        wt[:, :], c1_d[kt * P:(kt + 1) * P,
                                       half * CIN:(half + 1) * CIN])
                    c1ts.append(wt)
                for ch in range(CT // 2):
                    ct = half * (CT // 2) + ch
                    pa = psp.tile([P, N], F32, tag="acc", bufs=4)
                    pg = psp.tile([P, N], F32, tag="acc", bufs=4)
                    for kt in range(DT):
                        nc.tensor.matmul(pa[:, :], c1ts[kt][:, ch * P:ch * P + P],
                                         x2b[kt][:, :],
                                         start=(kt == 0), stop=(kt == DT - 1))
                    for kt in range(DT):
                        nc.tensor.matmul(pg[:, :],
                                         c1ts[kt][:, (CT // 2 + ch) * P:
                                                  (CT // 2 + ch) * P + P],
                                         x2b[kt][:, :],
                                         start=(kt == 0), stop=(kt == DT - 1))
                    # glu = (a + ba) * sigmoid(g) = 0.5*(a + ba)*(1 + tanh(g/2))
                    th = sb.tile([P, N], F32, tag="tmp", bufs=3)
                    nc.scalar.activation(th[:, :], pg[:, :], AF.Tanh,
                                         bias=c1ght[:, ct:ct + 1], scale=0.5)
                    u = sb.tile([P, N], F32, tag="glu_u", bufs=3)
                    nc.vector.tensor_scalar(out=u[:, :], in0=pa[:, :],
                                            scalar1=c1at[:, ct:ct + 1],
                                            scalar2=0.5,
                                            op0=AL.add, op1=AL.mult)
                    glu = sb.tile([P, PAD + N], BF16, tag="glu", bufs=5)
                    nc.vector.memset(glu[:, 0:PAD], 0.0)
                    nc.vector.scalar_tensor_tensor(glu[:, PAD:PAD + N], th[:, :],
                                                   1.0, u[:, :],
                                                   AL.add, AL.mult)
                    glus.append(glu)

            # depthwise conv as 31 diagonal matmuls per channel block; the
            # diagonal stationaries are built on-chip (DVE) from compact weights
            hcs = []
            for ct in range(CT):
                if ct not in dg_tiles:
                    build_dg(ct)
                dg = dg_tiles.pop(ct)
                pd = psp.tile([P, N], F32, tag="mm", bufs=2)
                for k in range(KW):
                    nc.tensor.matmul(pd[:, :], dg[:, k * P:(k + 1) * P],
                                     glus[ct][:, k:k + N],
                                     start=(k == 0), stop=(k == KW - 1))
                if ct + 2 < CT and (ct + 2) not in dg_tiles:
                    build_dg(ct + 2)
                # BN(eval) + swish in one ACT op: silu(s*x + t)
                hc = sb.tile([P, N], BF16, tag="hc", bufs=6)
                nc.scalar.activation(hc[:, :], pd[:, :], AF.Silu,
                                     bias=bntt[:, ct:ct + 1],
                                     scale=bnst[:, ct:ct + 1])
                hcs.append(hc)

            # conv2 + residual (kt-outer)
            pcs = [psp.tile([P, N], F32, tag="acc", bufs=4, name=f"pcs{i}") for i in range(DT)]
            for kt in range(CT):
                wt = sb.tile([P, DIM], BF16, tag="wsm", bufs=4)
                nc.sync.dma_start(wt[:, :], c2_d[kt * P:(kt + 1) * P, :])
                for mt in range(DT):
                    nc.tensor.matmul(pcs[mt][:, :], wt[:, mt * P:(mt + 1) * P],
                                     hcs[kt][:, :],
                                     start=(kt == 0), stop=(kt == CT - 1))
            x3 = residual(pcs, c2bt, x2)
            fill(4, tag="s1")

            # ================= ff2 =================
            x4 = ff_block(x3, w3_d, b3t, w4_d, b4t)

            # ================= post-LN =================
            m_b, r_b = layer_norm_rc(x4)
            for mt in range(DT):
                eng = nc.vector if mt % 2 == 0 else nc.gpsimd
                t = sb.tile([P, N], F32, tag="lnt", bufs=2)
                eng.tensor_sub(t[:, :], x4[mt][:, :], m_b[:, :])
                t2 = sb.tile([P, N], F32, tag="lnt2", bufs=2)
                eng.tensor_mul(t2[:, :], t[:, :], r_b[:, :])
                ot = sb.tile([P, N], F32, tag="outt", bufs=2)
                nc.vector.tensor_scalar(out=ot[:, :], in0=t2[:, :],
                                        scalar1=pngt[:, mt:mt + 1],
                                        scalar2=pnbt[:, mt:mt + 1],
                                        op0=AL.mult, op1=AL.add)
                nc.scalar.dma_start(outT_d[mt * P:(mt + 1) * P, :], ot[:, :])

    if split_waits:
        _split_matmul_waits(nc, mybir)
    return nc


def _split_matmul_waits(nc, mybir):
    """This walrus build rejects engine instructions carrying more than one
    sync wait; hoist the extras onto EventSemaphore instructions on the same
    engine queue right before the instruction."""
    fn = nc.m.functions[0]
    ctr = 0
    for blk in fn.blocks:
        out = []
        changed = False
        for ins in blk.instructions:
            si = ins.sync_info
            if (si is not None and si.on_wait and len(si.on_wait) > 1
                    and not isinstance(ins, (mybir.InstEventSemaphore,
                                             mybir.InstNoOp))):
                waits = list(si.on_wait)
                for w in waits[:-1]:
                    ev = mybir.InstNoOp(
                        name=f"EVW-{ctr}", ins=[], outs=[],
                        sync_info=mybir.SyncInfo(on_wait=[w], on_update=[]))
                    ev.engine = ins.engine
                    ctr += 1
                    out.append(ev)
                ins.sync_info = mybir.SyncInfo(
                    on_wait=[waits[-1]], on_update=list(si.on_update or []))
                changed = True
            out.append(ins)
        if changed:
            blk.instructions = out


def prep_inputs(inputs):
    """Host-side preprocessing: fold LN affines / scales / biases into weights."""
    import ml_dtypes

    f = np.float32
    bf = ml_dtypes.bfloat16
    ii = {k: np.asarray(v, dtype=f) for k, v in inputs.items()}

    def colmaj(b, nb):
        return np.ascontiguousarray(b.astype(f).reshape(nb, P).T)

    g1, be1 = ii["ff1_ln_g"], ii["ff1_ln_b"]
    w1 = np.ascontiguousarray((g1[:, None] * ii["ff1_w1"]).astype(bf))
    b1 = colmaj(be1 @ ii["ff1_w1"] + ii["ff1_b1"], FT)
    w2 = np.ascontiguousarray((0.5 * ii["ff1_w2"]).astype(bf))
    b2 = colmaj(0.5 * ii["ff1_b2"], DT)

    ag, ab = ii["attn_ln_g"], ii["attn_ln_b"]
    sc = DH ** -0.5
    wq = np.ascontiguousarray((ag[:, None] * ii["wq"] * sc).astype(bf))
    bq = colmaj((ab @ ii["wq"] + ii["bq"]) * sc, DT)
    wkv, bkv = ii["wkv"], ii["bkv"]
    # k bias is softmax-invariant (adds a per-query constant) -> dropped.
    wk = np.ascontiguousarray((ag[:, None] * wkv[:, :INNER]).astype(bf))
    wv = np.ascontiguousarray((ag[:, None] * wkv[:, INNER:]).astype(bf))
    bv = ab @ wkv[:, INNER:] + bkv[INNER:]
    wo = np.ascontiguousarray(ii["wo"].astype(bf))
    # v bias folded into the out-proj bias (attn rows sum to 1).
    bo = colmaj(ii["bo"] + bv @ ii["wo"], DT)
    # relT rows: head feature d lives at partition (h%2)*64 + d -> duplicate rows
    rT = ii["rel_emb"].T[:, ::-1]  # [64, 1025] column-reversed
    relT = np.ascontiguousarray(
        np.concatenate([rT, rT], axis=0).astype(bf))  # [128, 1025]

    # c1 columns reordered to match the kernel's half-split loop:
    # half h covers channel blocks ct=4h..4h+3 and lays out [a-cols | g-cols]
    w = ii["conv1_w"]
    HC = CIN // 2  # 512
    c1 = np.ascontiguousarray(np.concatenate(
        [w[:, 0:HC], w[:, CIN:CIN + HC], w[:, HC:CIN], w[:, CIN + HC:]],
        axis=1).astype(bf))
    c1b = ii["conv1_b"]
    c1a = colmaj(c1b[:CIN], CT)
    c1gh = colmaj(0.5 * c1b[CIN:], CT)
    # compact depthwise weights: dww[p, ct*KW + k] = dw_w[ct*128 + p, k]
    wr = ii["dw_w"].reshape(CT, P, KW)
    dww = np.ascontiguousarray(
        np.transpose(wr, (1, 0, 2)).reshape(P, CT * KW).astype(f))
    inv = 1.0 / np.sqrt(ii["bn_var"] + EPS)
    s = inv * ii["bn_g"]
    t = ii["bn_b"] - ii["bn_mean"] * s
    bns = colmaj(s, CT)
    bnt = colmaj(t + s * ii["dw_b"], CT)
    c2 = np.ascontiguousarray(ii["conv2_w"].astype(bf))
    c2b = colmaj(ii["conv2_b"], DT)

    g3, be3 = ii["ff2_ln_g"], ii["ff2_ln_b"]
    w3 = np.ascontiguousarray((g3[:, None] * ii["ff2_w1"]).astype(bf))
    b3 = colmaj(be3 @ ii["ff2_w1"] + ii["ff2_b1"], FT)
    w4 = np.ascontiguousarray((0.5 * ii["ff2_w2"]).astype(bf))
    b4 = colmaj(0.5 * ii["ff2_b2"], DT)

    png = colmaj(ii["pn_g"], DT)
    pnb = colmaj(ii["pn_b"], DT)

    shared = dict(w1=w1, b1=b1, w2=w2, b2=b2, wq=wq, bq=bq, wk=wk,
                  wv=wv, wo=wo, bo=bo, relT=relT, c1=c1, c1a=c1a,
                  c1gh=c1gh, dww=dww, bns=bns, bnt=bnt, c2=c2, c2b=c2b,
                  w3=w3, b3=b3, w4=w4, b4=b4, png=png, pnb=pnb,
                  antid=np.ascontiguousarray(np.eye(P, dtype=f)),
                  antidb=np.ascontiguousarray(np.eye(P, dtype=bf)),
                  onesf=np.ones((P, P), dtype=f))
    x = ii["x"]
    in_maps = []
    for b in range(NCORES):
        m = dict(shared)
        m["xT"] = np.ascontiguousarray(x[b].T)
        in_maps.append(m)
    return in_maps


_BUILT = None


def run(inputs, trace=False):
    global _BUILT
    from concourse import bass_utils

    in_maps = prep_inputs(inputs)
    if _BUILT is None:
        _BUILT = build()
    res = bass_utils.run_bass_kernel_spmd(
        _BUILT, in_maps, core_ids=list(range(NCORES)), trace=trace)
    out = np.stack([np.asarray(r["outT"]).T for r in res.results])
    return np.ascontiguousarray(out.astype(np.float32)), res


def kernel(**inputs):
    out, _ = run(inputs, trace=False)
    return out
